# revision 1
# baseline (speedup 1.0000x reference)
"""Multi-head self-attention Trainium2 kernel (8 NeuronCores).

Problem: x[4, 2048, 1024], H=16 heads, D=64. Sharding: core c handles
batch b = c // 2 and head-group hg = c % 2 (8 heads = 512 features).

Per-core math (F = 512 core-local features, T = 2048 tokens, C = 1024):
  QT = (Wq_s.T @ x_b.T) + bq_s          [F, T]   (feature-major)
  KT = same with Wk_s                    [F, T]
  V  = x_b @ Wv_s + bv_s                 [T, F]   (token-major)
  per head h (64-feature slice):
    scT[j, i] = KT_h.T-slices @ QT_h     (lhsT=KT tile, rhs=QT chunk)
    expT = exp(scT / 8)                  (no max subtraction: |s/8| < ~3)
    pv[0:64, i] = sum_j V65_h[j].T @ expT[j, i]   (V65 = [V_h | ones])
    pv[64, i]   = softmax denominator
    attnT_h[:, i] = pv[0:64, i] * (1 / pv[64, i])  (replicated via PE)
  o_part = attnT.T @ Wo_s                [T, C]
Host: out[b] = o_part[2b] + o_part[2b+1] + bo.
"""

import os
import sys

import numpy as np

if "/opt/trn_rl_repo" not in sys.path:
    sys.path.insert(0, "/opt/trn_rl_repo")

import concourse.bass as bass
import concourse.mybir as mybir
import concourse.tile as tile
from concourse import bacc

F32 = mybir.dt.float32
F32R = mybir.dt.float32r
AF = mybir.ActivationFunctionType

# Full-problem constants
B, N, C, H, D = 4, 2048, 1024, 16, 64
NCORES = 8
NH = 8          # heads per core
F = NH * D      # 512 core-local features
SCALE = 1.0 / 8.0  # 1/sqrt(D)


import ml_dtypes

BF16 = mybir.dt.bfloat16


def build_attention_kernel(tok=N, cin=C, nh=NH, mm_dt=F32R, debug=False):
    """Build the per-core Bass program. Returns the finalized Bass object.

    tok: sequence length, cin: model dim (= Wo output dim), nh: heads/core.
    """
    f = nh * D
    assert tok % 512 == 0 and cin % 128 == 0 and f % 128 == 0
    c_t = cin // 128       # contraction tiles for projections
    f_t = f // 128         # feature tiles (Q/K partition tiles)
    t_t = tok // 128       # token tiles
    iw = min(1024, tok)    # exp width (psum banks spanned = iw/512)
    n_ic = tok // iw
    nsub = iw // 512
    ocw = min(512, cin)    # output-proj column chunk width
    n_oc = cin // ocw

    nc = bacc.Bacc("TRN2", target_bir_lowering=False, debug=False,
                   num_devices=NCORES)

    xT = nc.dram_tensor("xT", [cin, tok], mm_dt, kind="ExternalInput").ap()
    wq = nc.dram_tensor("wq", [cin, f], mm_dt, kind="ExternalInput").ap()
    wk = nc.dram_tensor("wk", [cin, f], mm_dt, kind="ExternalInput").ap()
    wv = nc.dram_tensor("wv", [cin, f], mm_dt, kind="ExternalInput").ap()
    bq = nc.dram_tensor("bq", [f, 1], F32, kind="ExternalInput").ap()
    bk = nc.dram_tensor("bk", [f, 1], F32, kind="ExternalInput").ap()
    bv = nc.dram_tensor("bv", [1, f], mm_dt, kind="ExternalInput").ap()
    wo = nc.dram_tensor("wo", [f, cin], mm_dt, kind="ExternalInput").ap()
    o_part = nc.dram_tensor("o_part", [tok, cin], F32,
                            kind="ExternalOutput").ap()
    dbg = {}
    if debug:
        for nm, shp in (("d_qt", [128, tok]), ("d_kt", [128, tok]),
                        ("d_v", [128, nh * 65]), ("d_ex", [128, min(1024, tok)]),
                        ("d_pv", [65, 512]), ("d_at", [128, tok])):
            dbg[nm] = nc.dram_tensor(nm, shp, F32, kind="ExternalOutput").ap()

    with tile.TileContext(nc) as tc:
        from contextlib import ExitStack
        with ExitStack() as ctx:
            # ---- persistent pools (live whole kernel) ----
            p_qk = ctx.enter_context(tc.tile_pool(name="p_qk", bufs=1))
            p_v = ctx.enter_context(tc.tile_pool(name="p_v", bufs=1))
            p_sm = ctx.enter_context(tc.tile_pool(name="p_sm", bufs=1))
            ps_wide = ctx.enter_context(
                tc.tile_pool(name="ps_wide", bufs=2, space="PSUM"))
            ps_bank = ctx.enter_context(
                tc.tile_pool(name="ps_bank", bufs=3, space="PSUM"))

            QT = [p_qk.tile([128, tok], mm_dt, tag=f"qt{i}", name=f"QT{i}")
                  for i in range(f_t)]
            KT = [p_qk.tile([128, tok], mm_dt, tag=f"kt{i}", name=f"KT{i}")
                  for i in range(f_t)]
            # V65: per token-tile, per head 64 V columns + a ones column
            V65 = [p_v.tile([128, nh * 65], mm_dt, tag=f"v{i}", name=f"V65_{i}")
                   for i in range(t_t)]
            ones64 = p_sm.tile([1, 64], F32, tag="ones64", name="ones64")
            nc.vector.memset(ones64[:, :], 1.0)
            onesf = p_sm.tile([128, 128], F32, tag="onesf", name="onesf")
            nc.vector.memset(onesf[:, :], 1.0)
            onestok = p_sm.tile([1, 128], mm_dt, tag="onestok", name="onestok")
            nc.vector.tensor_copy(onestok[:, :], onesf[0:1, :])
            bqs = p_sm.tile([128, f_t], F32, tag="bqs", name="bqs")
            bks = p_sm.tile([128, f_t], F32, tag="bks", name="bks")
            bvs = p_sm.tile([1, f], mm_dt, tag="bvs", name="bvs")
            nc.sync.dma_start(bqs[:, :], bq.rearrange("(a p) o -> p (a o)", p=128))
            nc.sync.dma_start(bks[:, :], bk.rearrange("(a p) o -> p (a o)", p=128))
            nc.sync.dma_start(bvs[:, :], bv[:, :])

            # ================= Phase 1: Q/K/V projections =================
            with ExitStack() as ph1:
                p_w = ph1.enter_context(tc.tile_pool(name="p_w", bufs=1))
                p_xt = ph1.enter_context(tc.tile_pool(name="p_xt", bufs=2))

                wq_s = [p_w.tile([128, f], mm_dt, tag=f"wq{i}", name=f"wq_s{i}")
                        for i in range(c_t)]
                wk_s = [p_w.tile([128, f], mm_dt, tag=f"wk{i}", name=f"wk_s{i}")
                        for i in range(c_t)]
                wv_s = [p_w.tile([128, f], mm_dt, tag=f"wv{i}", name=f"wv_s{i}")
                        for i in range(c_t)]
                for i in range(c_t):
                    nc.sync.dma_start(wq_s[i][:, :], wq[i * 128:(i + 1) * 128, :])
                    nc.sync.dma_start(wk_s[i][:, :], wk[i * 128:(i + 1) * 128, :])
                    nc.sync.dma_start(wv_s[i][:, :], wv[i * 128:(i + 1) * 128, :])

                for tch in range(tok // 512):
                    ts = slice(tch * 512, (tch + 1) * 512)
                    xts = []
                    for i in range(c_t):
                        xt_i = p_xt.tile([128, 512], mm_dt, tag=f"x{i}",
                                         name=f"xt{i}_{tch}")
                        nc.sync.dma_start(xt_i[:, :], xT[i * 128:(i + 1) * 128, ts])
                        xts.append(xt_i)
                    # QT / KT feature-major tiles
                    for (w_s, dst, bias) in ((wq_s, QT, bqs), (wk_s, KT, bks)):
                        for ft in range(f_t):
                            ps = ps_wide.tile([128, 512], F32, tag="sc",
                                              name=f"psqk{ft}_{tch}")
                            for i in range(c_t):
                                nc.tensor.matmul(
                                    ps[:, :],
                                    w_s[i][:, ft * 128:(ft + 1) * 128],
                                    xts[i][:, :],
                                    start=(i == 0), stop=(i == c_t - 1))
                            nc.vector.tensor_scalar_add(
                                dst[ft][:, ts], ps[:, :], bias[:, ft:ft + 1])
                    # V token-major + bias via ones-row matmul
                    for tt4 in range(4):
                        gt = tch * 4 + tt4  # global token tile
                        tsl = slice(tt4 * 128, (tt4 + 1) * 128)
                        psv = ps_bank.tile([128, f], F32, tag="pv",
                                           name=f"psv{gt}")
                        for i in range(c_t):
                            nc.tensor.matmul(
                                psv[:, :], xts[i][:, tsl], wv_s[i][:, :],
                                start=(i == 0), stop=False)
                        nc.tensor.matmul(psv[:, :], onestok[:, :],
                                         bvs[:, :], start=False, stop=True)
                        v_dst = V65[gt].rearrange("p (h e) -> p h e", e=65)
                        nc.vector.tensor_copy(v_dst[:, :, 64:65],
                                              onesf[:, 0:nh])
                        nc.vector.tensor_copy(
                            v_dst[:, :, 0:64],
                            psv.rearrange("p (h e) -> p h e", e=64)[:, :, :])

            if debug:
                nc.sync.dma_start(dbg["d_qt"][:, :], QT[0][:, :].bitcast(F32))
                nc.sync.dma_start(dbg["d_kt"][:, :], KT[0][:, :].bitcast(F32))
                nc.sync.dma_start(dbg["d_v"][:, :], V65[0][:, :].bitcast(F32))

            # ================= Phase 2: attention =================
            p_at = ctx.enter_context(tc.tile_pool(name="p_at", bufs=1))
            p_wo = ctx.enter_context(tc.tile_pool(name="p_wo", bufs=1))
            p_ex = ctx.enter_context(tc.tile_pool(name="p_ex", bufs=3))
            p_dn = ctx.enter_context(tc.tile_pool(name="p_dn", bufs=2))
            p_os = ctx.enter_context(tc.tile_pool(name="p_os", bufs=3))

            attnT = [p_at.tile([128, tok], mm_dt, tag=f"at{i}", name=f"attnT{i}")
                     for i in range(f_t)]
            wo_s = [p_wo.tile([128, cin], mm_dt, tag=f"wo{i}", name=f"wo_s{i}")
                    for i in range(f_t)]
            for i in range(f_t):
                nc.sync.dma_start(wo_s[i][:, :], wo[i * 128:(i + 1) * 128, :])

            for h in range(nh):
                ft, r0 = h // 2, (h % 2) * 64
                kq_rows = slice(r0, r0 + 64)
                vcol = slice(h * 65, h * 65 + 65)
                for ic in range(n_ic):
                    pvs = [ps_bank.tile([65, 512], F32, tag="pv",
                                        name=f"pv{h}_{ic}_{s}")
                           for s in range(nsub)]
                    for jt in range(t_t):
                        sc = ps_wide.tile([128, iw], F32, tag="sc",
                                          name=f"sc{h}_{ic}_{jt}")
                        for s in range(nsub):
                            i0 = ic * iw + s * 512
                            nc.tensor.matmul(
                                sc[:, s * 512:(s + 1) * 512],
                                KT[ft][kq_rows, jt * 128:(jt + 1) * 128],
                                QT[ft][kq_rows, i0:i0 + 512],
                                start=True, stop=True)
                        ex = p_ex.tile([128, iw], mm_dt, tag="ex",
                                       name=f"ex{h}_{ic}_{jt}")
                        nc.scalar.activation(ex[:, :], sc[:, :], AF.Exp,
                                             scale=SCALE)
                        if debug and h == 0 and ic == 0 and jt == 0:
                            nc.sync.dma_start(dbg["d_ex"][:, :], ex[:, :].bitcast(F32))
                        for s in range(nsub):
                            nc.tensor.matmul(
                                pvs[s][:, :], V65[jt][:, vcol],
                                ex[:, s * 512:(s + 1) * 512],
                                start=(jt == 0), stop=(jt == t_t - 1))
                    if debug and h == 0 and ic == 0:
                        dpv = p_dn.tile([65, 512], F32, tag="dpv", name="dpv")
                        nc.vector.tensor_copy(dpv[:, :], pvs[0][:, :])
                        nc.sync.dma_start(dbg["d_pv"][:, :], dpv[:, :])
                    for s in range(nsub):
                        i0 = ic * iw + s * 512
                        isl = slice(i0, i0 + 512)
                        nc.vector.tensor_copy(attnT[ft][kq_rows, isl],
                                              pvs[s][0:64, :])
                        dn = p_dn.tile([1, 512], F32, tag="dn",
                                       name=f"dn{h}_{ic}_{s}")
                        nc.vector.tensor_copy(dn[:, :], pvs[s][64:65, :])
                        dninv = p_dn.tile([1, 512], F32, tag="dninv",
                                          name=f"dninv{h}_{ic}_{s}")
                        nc.vector.reciprocal_approx_fast(
                            out=dninv[:, :], in_=dn[:, :])
                        rp = ps_bank.tile([64, 512], F32, tag="pv",
                                          name=f"rp{h}_{ic}_{s}")
                        nc.tensor.matmul(rp[:, :], ones64[:, :], dninv[:, :],
                                         start=True, stop=True)
                        nc.vector.tensor_mul(attnT[ft][kq_rows, isl],
                                             attnT[ft][kq_rows, isl], rp[:, :])

            if debug:
                nc.sync.dma_start(dbg["d_at"][:, :], attnT[0][:, :].bitcast(F32))

            # ================= Phase 3: output projection =================
            for tt in range(t_t):
                tsl = slice(tt * 128, (tt + 1) * 128)
                for oc in range(n_oc):
                    osl = slice(oc * ocw, (oc + 1) * ocw)
                    po = ps_wide.tile([128, ocw], F32, tag="sc",
                                      name=f"po{tt}_{oc}")
                    for i in range(f_t):
                        nc.tensor.matmul(po[:, :], attnT[i][:, tsl],
                                         wo_s[i][:, osl],
                                         start=(i == 0), stop=(i == f_t - 1))
                    ob = p_os.tile([128, ocw], F32, tag="os",
                                   name=f"ob{tt}_{oc}")
                    nc.vector.tensor_copy(ob[:, :], po[:, :])
                    nc.sync.dma_start(o_part[tsl, osl], ob[:, :])

    nc.finalize()
    return nc


_NC_CACHE = {}


def _get_nc(key=(N, C, NH, F32R)):
    if key not in _NC_CACHE:
        _NC_CACHE[key] = build_attention_kernel(*key)
    return _NC_CACHE[key]


def make_in_maps(x, Wq, bq, Wk, bk, Wv, bv, Wo):
    """Shard full inputs into 8 per-core input maps."""
    in_maps = []
    for c in range(NCORES):
        b, hg = divmod(c, 2)
        fs = slice(hg * F, (hg + 1) * F)
        in_maps.append({
            "xT": np.ascontiguousarray(x[b].T),
            "wq": np.ascontiguousarray(Wq[:, fs]),
            "wk": np.ascontiguousarray(Wk[:, fs]),
            "wv": np.ascontiguousarray(Wv[:, fs]),
            "bq": np.ascontiguousarray(bq[fs].reshape(F, 1)),
            "bk": np.ascontiguousarray(bk[fs].reshape(F, 1)),
            "bv": np.ascontiguousarray(bv[fs].reshape(1, F)),
            "wo": np.ascontiguousarray(Wo[fs, :]),
        })
    return in_maps


def kernel(x, Wq, bq, Wk, bk, Wv, bv, Wo, bo, **_unused):
    from concourse.bass_utils import run_bass_kernel_spmd

    arrs = [np.asarray(a, dtype=np.float32)
            for a in (x, Wq, bq, Wk, bk, Wv, bv, Wo, bo)]
    x, Wq, bq, Wk, bk, Wv, bv, Wo, bo = arrs

    nc = _get_nc()
    in_maps = make_in_maps(x, Wq, bq, Wk, bk, Wv, bv, Wo)
    res = run_bass_kernel_spmd(nc, in_maps, core_ids=list(range(NCORES)))

    out = np.empty((B, N, C), dtype=np.float32)
    for b in range(B):
        out[b] = res.results[2 * b]["o_part"] + res.results[2 * b + 1]["o_part"] + bo
    return out



# revision 3
# speedup vs baseline: 1.5524x; 1.5524x over previous
"""Multi-head self-attention Trainium2 kernel (8 NeuronCores).

Problem: x[4, 2048, 1024], H=16 heads, D=64. Sharding: core c handles
batch b = c // 2 and head-group hg = c % 2 (8 heads = 512 features).

All matmul operands are bf16 (shipped pre-converted from host); PSUM
accumulation stays fp32. Per-core math (F = 512 local features,
T = 2048 tokens, C = 1024):

  QT = (Wq_s.T @ x_b.T) + bq_s          [F, T]   feature-major, bf16
  KT = same with Wk_s                    [F, T]
  V65 = [x_b @ Wv_s + bv_s | 1]          [T, 8*(64+1)]  token-major
  per head-pair f (heads 2f, 2f+1 in partition halves of tile f):
    sc[:, 0:512]   = KT[f][0:64].T-tile  @ QT[f][0:64]    (PE rows 0-63)
    sc[:, 512:1024]= KT[f][64:128].T-tile@ QT[f][64:128]  (PE rows 64-127,
                     adjacent in program order -> concurrent row-groups)
    ex = exp(sc / 8) bf16                (one ACT op per head-pair tile)
    pvX[0:64] += V65_hX.T @ ex-half ; pvX[64] = softmax denominator
    renorm: dninv = 1/pv[64] (DVE), broadcast over 64 partitions
            (GpSimd partition_broadcast), attnT = pv * bcast (DVE)
  o_part = attnT.T @ Wo_s                [T, C]  fp32 out
Host: out[b] = o_part[2b] + o_part[2b+1] + bo.
"""

import sys

import numpy as np

if "/opt/trn_rl_repo" not in sys.path:
    sys.path.insert(0, "/opt/trn_rl_repo")

import ml_dtypes

import concourse.bass as bass
import concourse.mybir as mybir
import concourse.tile as tile
from concourse import bacc

F32 = mybir.dt.float32
BF16 = mybir.dt.bfloat16
AF = mybir.ActivationFunctionType

# Full-problem constants
B, N, C, H, D = 4, 2048, 1024, 16, 64
NCORES = 8
NH = 8          # heads per core
F = NH * D      # 512 core-local features
SCALE = 1.0 / 8.0  # 1/sqrt(D)


def build_attention_kernel(tok=N, cin=C, nh=NH):
    """Build the per-core Bass program. Returns the finalized Bass object."""
    f = nh * D
    c_t = cin // 128       # contraction tiles for projections (8)
    f_t = f // 128         # feature tiles = head pairs (4)
    t_t = tok // 128       # token tiles (16)
    n_qc = tok // 512      # query chunks (4)
    ocw = 512
    n_oc = cin // ocw      # output-proj column chunks (2)

    nc = bacc.Bacc("TRN2", target_bir_lowering=False, debug=False,
                   num_devices=NCORES)

    xT = nc.dram_tensor("xT", [cin, tok], BF16, kind="ExternalInput").ap()
    wq = nc.dram_tensor("wq", [cin, f], BF16, kind="ExternalInput").ap()
    wk = nc.dram_tensor("wk", [cin, f], BF16, kind="ExternalInput").ap()
    wv = nc.dram_tensor("wv", [cin, f], BF16, kind="ExternalInput").ap()
    bq = nc.dram_tensor("bq", [f, 1], F32, kind="ExternalInput").ap()
    bk = nc.dram_tensor("bk", [f, 1], F32, kind="ExternalInput").ap()
    bv = nc.dram_tensor("bv", [1, f], BF16, kind="ExternalInput").ap()
    wo = nc.dram_tensor("wo", [f, cin], BF16, kind="ExternalInput").ap()
    o_part = nc.dram_tensor("o_part", [tok, cin], F32,
                            kind="ExternalOutput").ap()

    with tile.TileContext(nc) as tc:
        from contextlib import ExitStack
        with ExitStack() as ctx:
            # ---- persistent pools ----
            p_qk = ctx.enter_context(tc.tile_pool(name="p_qk", bufs=1))
            p_v = ctx.enter_context(tc.tile_pool(name="p_v", bufs=1))
            p_at = ctx.enter_context(tc.tile_pool(name="p_at", bufs=1))
            p_x = ctx.enter_context(tc.tile_pool(name="p_x", bufs=1))
            p_w = ctx.enter_context(tc.tile_pool(name="p_w", bufs=1))
            p_wo = ctx.enter_context(tc.tile_pool(name="p_wo", bufs=1))
            p_sm = ctx.enter_context(tc.tile_pool(name="p_sm", bufs=1))
            p_ex = ctx.enter_context(tc.tile_pool(name="p_ex", bufs=3))
            p_dn = ctx.enter_context(tc.tile_pool(name="p_dn", bufs=4))
            p_rb = ctx.enter_context(tc.tile_pool(name="p_rb", bufs=4))
            p_os = ctx.enter_context(tc.tile_pool(name="p_os", bufs=3))
            ps_sc = ctx.enter_context(
                tc.tile_pool(name="ps_sc", bufs=2, space="PSUM"))
            ps_pv = ctx.enter_context(
                tc.tile_pool(name="ps_pv", bufs=3, space="PSUM"))

            # ---- input DMAs ----
            xts = [p_x.tile([128, tok], BF16, tag=f"x{i}", name=f"xt{i}")
                   for i in range(c_t)]
            for i in range(c_t):
                nc.sync.dma_start(xts[i][:, :], xT[i * 128:(i + 1) * 128, :])
            wq_s = [p_w.tile([128, f], BF16, tag=f"wq{i}", name=f"wq_s{i}")
                    for i in range(c_t)]
            wk_s = [p_w.tile([128, f], BF16, tag=f"wk{i}", name=f"wk_s{i}")
                    for i in range(c_t)]
            wv_s = [p_w.tile([128, f], BF16, tag=f"wv{i}", name=f"wv_s{i}")
                    for i in range(c_t)]
            for i in range(c_t):
                nc.sync.dma_start(wq_s[i][:, :], wq[i * 128:(i + 1) * 128, :])
                nc.sync.dma_start(wk_s[i][:, :], wk[i * 128:(i + 1) * 128, :])
                nc.sync.dma_start(wv_s[i][:, :], wv[i * 128:(i + 1) * 128, :])
            wo_s = [p_wo.tile([128, cin], BF16, tag=f"wo{i}", name=f"wo_s{i}")
                    for i in range(f_t)]
            for i in range(f_t):
                nc.sync.dma_start(wo_s[i][:, :], wo[i * 128:(i + 1) * 128, :])

            bqs = p_sm.tile([128, f_t], F32, tag="bqs", name="bqs")
            bks = p_sm.tile([128, f_t], F32, tag="bks", name="bks")
            bvs = p_sm.tile([1, f], BF16, tag="bvs", name="bvs")
            nc.sync.dma_start(bqs[:, :], bq.rearrange("(a p) o -> p (a o)", p=128))
            nc.sync.dma_start(bks[:, :], bk.rearrange("(a p) o -> p (a o)", p=128))
            nc.sync.dma_start(bvs[:, :], bv[:, :])

            onesf = p_sm.tile([128, nh], BF16, tag="onesf", name="onesf")
            nc.vector.memset(onesf[:, :], 1.0)
            onestok = p_sm.tile([1, 128], BF16, tag="onestok", name="onestok")
            nc.vector.memset(onestok[:, :], 1.0)

            QT = [p_qk.tile([128, tok], BF16, tag=f"qt{i}", name=f"QT{i}")
                  for i in range(f_t)]
            KT = [p_qk.tile([128, tok], BF16, tag=f"kt{i}", name=f"KT{i}")
                  for i in range(f_t)]
            V65 = [p_v.tile([128, nh * 65], BF16, tag=f"v{i}", name=f"V65_{i}")
                   for i in range(t_t)]
            attnT = [p_at.tile([128, tok], BF16, tag=f"at{i}", name=f"attnT{i}")
                     for i in range(f_t)]

            def emit_qk(ft):
                """QT[ft], KT[ft] (feature-major) over all token chunks."""
                for tch in range(tok // 512):
                    ts = slice(tch * 512, (tch + 1) * 512)
                    for (w_s, dst, bias) in ((wq_s, QT, bqs), (wk_s, KT, bks)):
                        ps = ps_sc.tile([128, 512], F32, tag="sc",
                                        name=f"psqk{ft}_{tch}_{dst[ft].name}")
                        for i in range(c_t):
                            nc.tensor.matmul(
                                ps[:, :],
                                w_s[i][:, ft * 128:(ft + 1) * 128],
                                xts[i][:, ts],
                                start=(i == 0), stop=(i == c_t - 1))
                        nc.vector.tensor_scalar_add(
                            dst[ft][:, ts], ps[:, :], bias[:, ft:ft + 1])

            def emit_v(gt):
                """V65[gt]: token-major V + bias + ones column, bf16."""
                tsl = slice(gt * 128, (gt + 1) * 128)
                psv = ps_pv.tile([128, f], F32, tag="pv", name=f"psv{gt}")
                for i in range(c_t):
                    nc.tensor.matmul(
                        psv[:, :], xts[i][:, tsl], wv_s[i][:, :],
                        start=(i == 0), stop=False)
                nc.tensor.matmul(psv[:, :], onestok[:, :], bvs[:, :],
                                 start=False, stop=True)
                v_dst = V65[gt].rearrange("p (h e) -> p h e", e=65)
                nc.vector.tensor_copy(v_dst[:, :, 64:65], onesf[:, 0:nh])
                nc.vector.tensor_copy(
                    v_dst[:, :, 0:64],
                    psv.rearrange("p (h e) -> p h e", e=64)[:, :, :])

            def emit_attn(ft):
                """Attention for head pair ft (heads 2ft, 2ft+1)."""
                hA, hB = 2 * ft, 2 * ft + 1
                for qc in range(n_qc):
                    qs = slice(qc * 512, (qc + 1) * 512)
                    pvA = ps_pv.tile([65, 512], F32, tag="pv",
                                     name=f"pvA{ft}_{qc}")
                    pvB = ps_pv.tile([65, 512], F32, tag="pv",
                                     name=f"pvB{ft}_{qc}")
                    for jt in range(t_t):
                        js = slice(jt * 128, (jt + 1) * 128)
                        sc = ps_sc.tile([128, 1024], F32, tag="sc",
                                        name=f"sc{ft}_{qc}_{jt}")
                        # paired score matmuls in disjoint PE row groups
                        nc.tensor.matmul(sc[:, 0:512],
                                         KT[ft][0:64, js], QT[ft][0:64, qs],
                                         start=True, stop=True)
                        nc.tensor.matmul(sc[:, 512:1024],
                                         KT[ft][64:128, js], QT[ft][64:128, qs],
                                         start=True, stop=True)
                        ex = p_ex.tile([128, 1024], BF16, tag="ex",
                                       name=f"ex{ft}_{qc}_{jt}")
                        nc.scalar.activation(ex[:, :], sc[:, :], AF.Exp,
                                             scale=SCALE)
                        nc.tensor.matmul(pvA[:, :],
                                         V65[jt][:, hA * 65:(hA + 1) * 65],
                                         ex[:, 0:512],
                                         start=(jt == 0), stop=(jt == t_t - 1))
                        nc.tensor.matmul(pvB[:, :],
                                         V65[jt][:, hB * 65:(hB + 1) * 65],
                                         ex[:, 512:1024],
                                         start=(jt == 0), stop=(jt == t_t - 1))
                    for (pv, r0) in ((pvA, 0), (pvB, 64)):
                        dn = p_dn.tile([1, 512], F32, tag="dn",
                                       name=f"dn{ft}_{qc}_{r0}")
                        nc.vector.tensor_copy(dn[:, :], pv[64:65, :])
                        dninv = p_dn.tile([1, 512], F32, tag="dninv",
                                          name=f"dninv{ft}_{qc}_{r0}")
                        nc.vector.reciprocal_approx_fast(
                            out=dninv[:, :], in_=dn[:, :])
                        rpb = p_rb.tile([64, 512], F32, tag="rpb",
                                        name=f"rpb{ft}_{qc}_{r0}")
                        nc.gpsimd.partition_broadcast(rpb[:, :], dninv[:, :])
                        nc.vector.tensor_mul(attnT[ft][r0:r0 + 64, qs],
                                             pv[0:64, :], rpb[:, :])

            # ---- emission order: QK0 first so attention starts early;
            # V65 next; then attn[f] with QK[f+1] as PE gap-filler.
            emit_qk(0)
            for gt in range(t_t):
                emit_v(gt)
            for ft in range(f_t):
                emit_attn(ft)
                if ft + 1 < f_t:
                    emit_qk(ft + 1)

            # ---- output projection ----
            for tt in range(t_t):
                tsl = slice(tt * 128, (tt + 1) * 128)
                for oc in range(n_oc):
                    osl = slice(oc * ocw, (oc + 1) * ocw)
                    po = ps_sc.tile([128, ocw], F32, tag="sc",
                                    name=f"po{tt}_{oc}")
                    for i in range(f_t):
                        nc.tensor.matmul(po[:, :], attnT[i][:, tsl],
                                         wo_s[i][:, osl],
                                         start=(i == 0), stop=(i == f_t - 1))
                    ob = p_os.tile([128, ocw], F32, tag="os",
                                   name=f"ob{tt}_{oc}")
                    nc.vector.tensor_copy(ob[:, :], po[:, :])
                    nc.sync.dma_start(o_part[tsl, osl], ob[:, :])

    nc.finalize()
    return nc


_NC_CACHE = {}


def _get_nc(key=(N, C, NH)):
    if key not in _NC_CACHE:
        _NC_CACHE[key] = build_attention_kernel(*key)
    return _NC_CACHE[key]


def make_in_maps(x, Wq, bq, Wk, bk, Wv, bv, Wo):
    """Shard full inputs into 8 per-core input maps (bf16 operands)."""
    bf = ml_dtypes.bfloat16
    in_maps = []
    for c in range(NCORES):
        b, hg = divmod(c, 2)
        fs = slice(hg * F, (hg + 1) * F)
        in_maps.append({
            "xT": np.ascontiguousarray(x[b].T.astype(bf)),
            "wq": np.ascontiguousarray(Wq[:, fs].astype(bf)),
            "wk": np.ascontiguousarray(Wk[:, fs].astype(bf)),
            "wv": np.ascontiguousarray(Wv[:, fs].astype(bf)),
            "bq": np.ascontiguousarray(bq[fs].reshape(F, 1)),
            "bk": np.ascontiguousarray(bk[fs].reshape(F, 1)),
            "bv": np.ascontiguousarray(bv[fs].reshape(1, F).astype(bf)),
            "wo": np.ascontiguousarray(Wo[fs, :].astype(bf)),
        })
    return in_maps


def kernel(x, Wq, bq, Wk, bk, Wv, bv, Wo, bo, **_unused):
    from concourse.bass_utils import run_bass_kernel_spmd

    arrs = [np.asarray(a, dtype=np.float32)
            for a in (x, Wq, bq, Wk, bk, Wv, bv, Wo, bo)]
    x, Wq, bq, Wk, bk, Wv, bv, Wo, bo = arrs

    nc = _get_nc()
    in_maps = make_in_maps(x, Wq, bq, Wk, bk, Wv, bv, Wo)
    res = run_bass_kernel_spmd(nc, in_maps, core_ids=list(range(NCORES)))

    out = np.empty((B, N, C), dtype=np.float32)
    for b in range(B):
        out[b] = res.results[2 * b]["o_part"] + res.results[2 * b + 1]["o_part"] + bo
    return out


# revision 4
# speedup vs baseline: 1.7803x; 1.1468x over previous
"""Multi-head self-attention Trainium2 kernel (8 NeuronCores).

Problem: x[4, 2048, 1024], H=16 heads, D=64. Sharding: core c handles
batch b = c // 2 and head-group hg = c % 2 (8 heads = 512 features).

All matmul operands are bf16 (shipped pre-converted from host); PSUM
accumulation stays fp32. Per-core math (F = 512 local features,
T = 2048 tokens, C = 1024):

  QT = (Wq_s.T @ x_b.T) + bq_s          [F, T]   feature-major, bf16
  KT = same with Wk_s                    [F, T]
  V65 = [x_b @ Wv_s + bv_s | 1]          [T, 8*(64+1)]  token-major
  per head-pair f (heads 2f, 2f+1 in partition halves of tile f):
    sc[:, 0:512]   = KT[f][0:64].T-tile  @ QT[f][0:64]    (PE rows 0-63)
    sc[:, 512:1024]= KT[f][64:128].T-tile@ QT[f][64:128]  (PE rows 64-127,
                     adjacent in program order -> concurrent row-groups)
    ex = exp(sc / 8) bf16                (one ACT op per head-pair tile)
    pvX[0:64] += V65_hX.T @ ex-half ; pvX[64] = softmax denominator
    renorm: dninv = 1/pv[64] (DVE), broadcast over 64 partitions
            (GpSimd partition_broadcast), attnT = pv * bcast (DVE)
  o_part = attnT.T @ Wo_s                [T, C]  fp32 out

QK projections for head-pair f+1 are emitted after attention f so the
scheduler threads them into PE gaps while ACT (exp) is the bottleneck;
they accumulate in a separate 1-bank PSUM tag to avoid slot contention
with the attention score tiles. The output projection is emitted per
query-chunk inside the last head-pair for the same reason.
Host: out[b] = o_part[2b] + o_part[2b+1] + bo.
"""

import sys

import numpy as np

if "/opt/trn_rl_repo" not in sys.path:
    sys.path.insert(0, "/opt/trn_rl_repo")

import ml_dtypes

import concourse.bass as bass
import concourse.mybir as mybir
import concourse.tile as tile
from concourse import bacc

F32 = mybir.dt.float32
BF16 = mybir.dt.bfloat16
AF = mybir.ActivationFunctionType

# Full-problem constants
B, N, C, H, D = 4, 2048, 1024, 16, 64
NCORES = 8
NH = 8          # heads per core
F = NH * D      # 512 core-local features
SCALE = 1.0 / 8.0  # 1/sqrt(D)


def build_attention_kernel(tok=N, cin=C, nh=NH):
    """Build the per-core Bass program. Returns the finalized Bass object."""
    f = nh * D
    c_t = cin // 128       # contraction tiles for projections (8)
    f_t = f // 128         # feature tiles = head pairs (4)
    t_t = tok // 128       # token tiles (16)
    n_qc = tok // 512      # query chunks (4)
    ocw = 512
    n_oc = cin // ocw      # output-proj column chunks (2)

    nc = bacc.Bacc("TRN2", target_bir_lowering=False, debug=False,
                   num_devices=NCORES)

    xT = nc.dram_tensor("xT", [cin, tok], BF16, kind="ExternalInput").ap()
    wq = nc.dram_tensor("wq", [cin, f], BF16, kind="ExternalInput").ap()
    wk = nc.dram_tensor("wk", [cin, f], BF16, kind="ExternalInput").ap()
    wv = nc.dram_tensor("wv", [cin, f], BF16, kind="ExternalInput").ap()
    bq = nc.dram_tensor("bq", [f, 1], F32, kind="ExternalInput").ap()
    bk = nc.dram_tensor("bk", [f, 1], F32, kind="ExternalInput").ap()
    bv = nc.dram_tensor("bv", [1, f], F32, kind="ExternalInput").ap()
    wo = nc.dram_tensor("wo", [f, cin], BF16, kind="ExternalInput").ap()
    o_part = nc.dram_tensor("o_part", [tok, cin], F32,
                            kind="ExternalOutput").ap()

    with tile.TileContext(nc) as tc:
        from contextlib import ExitStack
        with ExitStack() as ctx:
            # ---- persistent pools ----
            p_qk = ctx.enter_context(tc.tile_pool(name="p_qk", bufs=1))
            p_v = ctx.enter_context(tc.tile_pool(name="p_v", bufs=1))
            p_at = ctx.enter_context(tc.tile_pool(name="p_at", bufs=1))
            p_x = ctx.enter_context(tc.tile_pool(name="p_x", bufs=1))
            p_w = ctx.enter_context(tc.tile_pool(name="p_w", bufs=1))
            p_wo = ctx.enter_context(tc.tile_pool(name="p_wo", bufs=1))
            p_sm = ctx.enter_context(tc.tile_pool(name="p_sm", bufs=1))
            p_ex = ctx.enter_context(tc.tile_pool(name="p_ex", bufs=4))
            p_dn = ctx.enter_context(tc.tile_pool(name="p_dn", bufs=4))
            p_rb = ctx.enter_context(tc.tile_pool(name="p_rb", bufs=4))
            p_os = ctx.enter_context(tc.tile_pool(name="p_os", bufs=3))
            ps_sc = ctx.enter_context(
                tc.tile_pool(name="ps_sc", bufs=2, space="PSUM"))
            ps_pv = ctx.enter_context(
                tc.tile_pool(name="ps_pv", bufs=3, space="PSUM"))
            ps_pj = ctx.enter_context(
                tc.tile_pool(name="ps_pj", bufs=1, space="PSUM"))

            # ---- input DMAs (ordered so QK0 can start earliest) ----
            xts = [p_x.tile([128, tok], BF16, tag=f"x{i}", name=f"xt{i}")
                   for i in range(c_t)]
            wq_s = [p_w.tile([128, f], BF16, tag=f"wq{i}", name=f"wq_s{i}")
                    for i in range(c_t)]
            wk_s = [p_w.tile([128, f], BF16, tag=f"wk{i}", name=f"wk_s{i}")
                    for i in range(c_t)]
            wv_s = [p_w.tile([128, f], BF16, tag=f"wv{i}", name=f"wv_s{i}")
                    for i in range(c_t)]
            bqs = p_sm.tile([128, f_t], F32, tag="bqs", name="bqs")
            bks = p_sm.tile([128, f_t], F32, tag="bks", name="bks")
            bvs = p_sm.tile([1, f], F32, tag="bvs", name="bvs")
            nc.sync.dma_start(bqs[:, :], bq.rearrange("(a p) o -> p (a o)", p=128))
            nc.sync.dma_start(bks[:, :], bk.rearrange("(a p) o -> p (a o)", p=128))
            nc.sync.dma_start(bvs[:, :], bv[:, :])
            for i in range(c_t):
                nc.sync.dma_start(xts[i][:, :], xT[i * 128:(i + 1) * 128, :])
                nc.sync.dma_start(wq_s[i][:, :], wq[i * 128:(i + 1) * 128, :])
                nc.sync.dma_start(wk_s[i][:, :], wk[i * 128:(i + 1) * 128, :])
            for i in range(c_t):
                nc.sync.dma_start(wv_s[i][:, :], wv[i * 128:(i + 1) * 128, :])
            wo_s = [p_wo.tile([128, cin], BF16, tag=f"wo{i}", name=f"wo_s{i}")
                    for i in range(f_t)]
            for i in range(f_t):
                nc.sync.dma_start(wo_s[i][:, :], wo[i * 128:(i + 1) * 128, :])

            onesf = p_sm.tile([128, nh], BF16, tag="onesf", name="onesf")
            nc.vector.memset(onesf[:, :], 1.0)
            # bv broadcast over all token partitions (used in V65 cast)
            bvs_bc = p_sm.tile([128, f], F32, tag="bvsbc", name="bvs_bc")
            nc.gpsimd.partition_broadcast(bvs_bc[:, :], bvs[:, :])

            QT = [p_qk.tile([128, tok], BF16, tag=f"qt{i}", name=f"QT{i}")
                  for i in range(f_t)]
            KT = [p_qk.tile([128, tok], BF16, tag=f"kt{i}", name=f"KT{i}")
                  for i in range(f_t)]
            V65 = [p_v.tile([128, nh * 65], BF16, tag=f"v{i}", name=f"V65_{i}")
                   for i in range(t_t)]
            attnT = [p_at.tile([128, tok], BF16, tag=f"at{i}", name=f"attnT{i}")
                     for i in range(f_t)]

            def emit_qk(ft):
                """QT[ft], KT[ft] (feature-major) over all token chunks."""
                for tch in range(tok // 512):
                    ts = slice(tch * 512, (tch + 1) * 512)
                    for (w_s, dst, bias) in ((wq_s, QT, bqs), (wk_s, KT, bks)):
                        ps = ps_pj.tile([128, 512], F32, tag="pj",
                                        name=f"psqk{ft}_{tch}_{dst[ft].name}")
                        for i in range(c_t):
                            nc.tensor.matmul(
                                ps[:, :],
                                w_s[i][:, ft * 128:(ft + 1) * 128],
                                xts[i][:, ts],
                                start=(i == 0), stop=(i == c_t - 1))
                        nc.vector.tensor_scalar_add(
                            dst[ft][:, ts], ps[:, :], bias[:, ft:ft + 1])

            def emit_v(gt):
                """V65[gt]: token-major V + bias + ones column, bf16."""
                tsl = slice(gt * 128, (gt + 1) * 128)
                psv = ps_pv.tile([128, f], F32, tag="pv", name=f"psv{gt}")
                for i in range(c_t):
                    nc.tensor.matmul(
                        psv[:, :], xts[i][:, tsl], wv_s[i][:, :],
                        start=(i == 0), stop=(i == c_t - 1))
                v_dst = V65[gt].rearrange("p (h e) -> p h e", e=65)
                nc.vector.tensor_copy(v_dst[:, :, 64:65], onesf[:, 0:nh])
                nc.vector.tensor_add(
                    v_dst[:, :, 0:64],
                    psv.rearrange("p (h e) -> p h e", e=64)[:, :, :],
                    bvs_bc.rearrange("p (h e) -> p h e", e=64)[:, :, :])

            def emit_out_proj(tt):
                """Output projection for token tile tt (128 tokens)."""
                tsl = slice(tt * 128, (tt + 1) * 128)
                for oc in range(n_oc):
                    osl = slice(oc * ocw, (oc + 1) * ocw)
                    po = ps_pj.tile([128, ocw], F32, tag="pj",
                                    name=f"po{tt}_{oc}")
                    for i in range(f_t):
                        nc.tensor.matmul(po[:, :], attnT[i][:, tsl],
                                         wo_s[i][:, osl],
                                         start=(i == 0), stop=(i == f_t - 1))
                    ob = p_os.tile([128, ocw], F32, tag="os",
                                   name=f"ob{tt}_{oc}")
                    nc.vector.tensor_copy(ob[:, :], po[:, :])
                    nc.sync.dma_start(o_part[tsl, osl], ob[:, :])

            def emit_attn(ft):
                """Attention for head pair ft (heads 2ft, 2ft+1)."""
                hA, hB = 2 * ft, 2 * ft + 1
                for qc in range(n_qc):
                    qs = slice(qc * 512, (qc + 1) * 512)
                    pvA = ps_pv.tile([65, 512], F32, tag="pv",
                                     name=f"pvA{ft}_{qc}")
                    pvB = ps_pv.tile([65, 512], F32, tag="pv",
                                     name=f"pvB{ft}_{qc}")
                    for jt in range(t_t):
                        js = slice(jt * 128, (jt + 1) * 128)
                        sc = ps_sc.tile([128, 1024], F32, tag="sc",
                                        name=f"sc{ft}_{qc}_{jt}")
                        # paired score matmuls in disjoint PE row groups
                        nc.tensor.matmul(sc[:, 0:512],
                                         KT[ft][0:64, js], QT[ft][0:64, qs],
                                         start=True, stop=True)
                        nc.tensor.matmul(sc[:, 512:1024],
                                         KT[ft][64:128, js], QT[ft][64:128, qs],
                                         start=True, stop=True)
                        ex = p_ex.tile([128, 1024], BF16, tag="ex",
                                       name=f"ex{ft}_{qc}_{jt}")
                        nc.scalar.activation(ex[:, :], sc[:, :], AF.Exp,
                                             scale=SCALE)
                        nc.tensor.matmul(pvA[:, :],
                                         V65[jt][:, hA * 65:(hA + 1) * 65],
                                         ex[:, 0:512],
                                         start=(jt == 0), stop=(jt == t_t - 1))
                        nc.tensor.matmul(pvB[:, :],
                                         V65[jt][:, hB * 65:(hB + 1) * 65],
                                         ex[:, 512:1024],
                                         start=(jt == 0), stop=(jt == t_t - 1))
                    for (pv, r0) in ((pvA, 0), (pvB, 64)):
                        dn = p_dn.tile([1, 512], F32, tag="dn",
                                       name=f"dn{ft}_{qc}_{r0}")
                        nc.vector.tensor_copy(dn[:, :], pv[64:65, :])
                        dninv = p_dn.tile([1, 512], F32, tag="dninv",
                                          name=f"dninv{ft}_{qc}_{r0}")
                        nc.vector.reciprocal_approx_fast(
                            out=dninv[:, :], in_=dn[:, :])
                        rpb = p_rb.tile([64, 512], F32, tag="rpb",
                                        name=f"rpb{ft}_{qc}_{r0}")
                        nc.gpsimd.partition_broadcast(rpb[:, :], dninv[:, :])
                        nc.vector.tensor_mul(attnT[ft][r0:r0 + 64, qs],
                                             pv[0:64, :], rpb[:, :])
                    if ft == f_t - 1:
                        # all heads done for these 512 tokens: project out
                        for tt in range(qc * 4, (qc + 1) * 4):
                            emit_out_proj(tt)

            # ---- emission order: QK0 first so attention starts early;
            # V65 next; then attn[f] with QK[f+1] as PE gap-filler.
            emit_qk(0)
            for gt in range(t_t):
                emit_v(gt)
            for ft in range(f_t):
                emit_attn(ft)
                if ft + 1 < f_t:
                    emit_qk(ft + 1)

    nc.finalize()
    return nc


_NC_CACHE = {}


def _get_nc(key=(N, C, NH)):
    if key not in _NC_CACHE:
        _NC_CACHE[key] = build_attention_kernel(*key)
    return _NC_CACHE[key]


def make_in_maps(x, Wq, bq, Wk, bk, Wv, bv, Wo):
    """Shard full inputs into 8 per-core input maps (bf16 operands)."""
    bf = ml_dtypes.bfloat16
    in_maps = []
    for c in range(NCORES):
        b, hg = divmod(c, 2)
        fs = slice(hg * F, (hg + 1) * F)
        in_maps.append({
            "xT": np.ascontiguousarray(x[b].T.astype(bf)),
            "wq": np.ascontiguousarray(Wq[:, fs].astype(bf)),
            "wk": np.ascontiguousarray(Wk[:, fs].astype(bf)),
            "wv": np.ascontiguousarray(Wv[:, fs].astype(bf)),
            "bq": np.ascontiguousarray(bq[fs].reshape(F, 1)),
            "bk": np.ascontiguousarray(bk[fs].reshape(F, 1)),
            "bv": np.ascontiguousarray(bv[fs].reshape(1, F)),
            "wo": np.ascontiguousarray(Wo[fs, :].astype(bf)),
        })
    return in_maps


def kernel(x, Wq, bq, Wk, bk, Wv, bv, Wo, bo, **_unused):
    from concourse.bass_utils import run_bass_kernel_spmd

    arrs = [np.asarray(a, dtype=np.float32)
            for a in (x, Wq, bq, Wk, bk, Wv, bv, Wo, bo)]
    x, Wq, bq, Wk, bk, Wv, bv, Wo, bo = arrs

    nc = _get_nc()
    in_maps = make_in_maps(x, Wq, bq, Wk, bk, Wv, bv, Wo)
    res = run_bass_kernel_spmd(nc, in_maps, core_ids=list(range(NCORES)))

    out = np.empty((B, N, C), dtype=np.float32)
    for b in range(B):
        out[b] = res.results[2 * b]["o_part"] + res.results[2 * b + 1]["o_part"] + bo
    return out


# revision 5
# speedup vs baseline: 1.8364x; 1.0315x over previous
"""Multi-head self-attention Trainium2 kernel (8 NeuronCores).

Problem: x[4, 2048, 1024], H=16 heads, D=64. Sharding: core c handles
batch b = c // 2 and head-group hg = c % 2 (8 heads = 512 features).

All matmul operands are bf16 (shipped pre-converted from host); PSUM
accumulation stays fp32. Per-core math (F = 512 local features,
T = 2048 tokens, C = 1024):

  QT = (Wq_s.T @ x_b.T) + bq_s          [F, T]   feature-major, bf16
  KT = same with Wk_s                    [F, T]
  V65 = [x_b @ Wv_s + bv_s | 1]          [T, 8*(64+1)]  token-major
  per head-pair f (heads 2f, 2f+1 in partition halves of tile f):
    sc[:, 0:512]   = KT[f][0:64].T-tile  @ QT[f][0:64]    (PE rows 0-63)
    sc[:, 512:1024]= KT[f][64:128].T-tile@ QT[f][64:128]  (PE rows 64-127,
                     adjacent in program order -> concurrent row-groups)
    ex = exp(sc / 8) bf16                (one ACT op per head-pair tile)
    pvX[0:64] += V65_hX.T @ ex-half ; pvX[64] = softmax denominator
    renorm: dninv = 1/pv[64] (DVE), broadcast over 64 partitions
            (GpSimd partition_broadcast), attnT = pv * bcast (DVE)
  o_part = attnT.T @ Wo_s                [T, C]  fp32 out

QK projections for head-pair f+1 are emitted after attention f so the
scheduler threads them into PE gaps while ACT (exp) is the bottleneck;
they accumulate in a separate 1-bank PSUM tag to avoid slot contention
with the attention score tiles. The output projection is emitted per
query-chunk inside the last head-pair for the same reason.
Host: out[b] = o_part[2b] + o_part[2b+1] + bo.
"""

import sys

import numpy as np

if "/opt/trn_rl_repo" not in sys.path:
    sys.path.insert(0, "/opt/trn_rl_repo")

import ml_dtypes

import concourse.bass as bass
import concourse.mybir as mybir
import concourse.tile as tile
from concourse import bacc

F32 = mybir.dt.float32
BF16 = mybir.dt.bfloat16
AF = mybir.ActivationFunctionType

# Full-problem constants
B, N, C, H, D = 4, 2048, 1024, 16, 64
NCORES = 8
NH = 8          # heads per core
F = NH * D      # 512 core-local features
SCALE = 1.0 / 8.0  # 1/sqrt(D)


def build_attention_kernel(tok=N, cin=C, nh=NH):
    """Build the per-core Bass program. Returns the finalized Bass object."""
    f = nh * D
    c_t = cin // 128       # contraction tiles for projections (8)
    f_t = f // 128         # feature tiles = head pairs (4)
    t_t = tok // 128       # token tiles (16)
    n_qc = tok // 512      # query chunks (4)
    ocw = 512
    n_oc = cin // ocw      # output-proj column chunks (2)

    nc = bacc.Bacc("TRN2", target_bir_lowering=False, debug=False,
                   num_devices=NCORES)

    xT = nc.dram_tensor("xT", [cin, tok], BF16, kind="ExternalInput").ap()
    wq = nc.dram_tensor("wq", [cin, f], BF16, kind="ExternalInput").ap()
    wk = nc.dram_tensor("wk", [cin, f], BF16, kind="ExternalInput").ap()
    wv = nc.dram_tensor("wv", [cin, f], BF16, kind="ExternalInput").ap()
    bq = nc.dram_tensor("bq", [f, 1], F32, kind="ExternalInput").ap()
    bk = nc.dram_tensor("bk", [f, 1], F32, kind="ExternalInput").ap()
    bv = nc.dram_tensor("bv", [1, f], F32, kind="ExternalInput").ap()
    wo = nc.dram_tensor("wo", [f, cin], BF16, kind="ExternalInput").ap()
    o_part = nc.dram_tensor("o_part", [tok, cin], F32,
                            kind="ExternalOutput").ap()

    with tile.TileContext(nc) as tc:
        from contextlib import ExitStack
        with ExitStack() as ctx:
            # ---- persistent pools ----
            p_qk = ctx.enter_context(tc.tile_pool(name="p_qk", bufs=1))
            p_v = ctx.enter_context(tc.tile_pool(name="p_v", bufs=1))
            p_at = ctx.enter_context(tc.tile_pool(name="p_at", bufs=1))
            p_x = ctx.enter_context(tc.tile_pool(name="p_x", bufs=1))
            p_w = ctx.enter_context(tc.tile_pool(name="p_w", bufs=1))
            p_wo = ctx.enter_context(tc.tile_pool(name="p_wo", bufs=1))
            p_sm = ctx.enter_context(tc.tile_pool(name="p_sm", bufs=1))
            p_ex = ctx.enter_context(tc.tile_pool(name="p_ex", bufs=4))
            p_dn = ctx.enter_context(tc.tile_pool(name="p_dn", bufs=4))
            p_rb = ctx.enter_context(tc.tile_pool(name="p_rb", bufs=4))
            p_os = ctx.enter_context(tc.tile_pool(name="p_os", bufs=4))
            ps_sc = ctx.enter_context(
                tc.tile_pool(name="ps_sc", bufs=2, space="PSUM"))
            ps_pv = ctx.enter_context(
                tc.tile_pool(name="ps_pv", bufs=3, space="PSUM"))
            ps_pj = ctx.enter_context(
                tc.tile_pool(name="ps_pj", bufs=1, space="PSUM"))

            # ---- input DMAs (ordered so QK0 can start earliest) ----
            xts = [p_x.tile([128, tok], BF16, tag=f"x{i}", name=f"xt{i}")
                   for i in range(c_t)]
            wq_s = [p_w.tile([128, f], BF16, tag=f"wq{i}", name=f"wq_s{i}")
                    for i in range(c_t)]
            wk_s = [p_w.tile([128, f], BF16, tag=f"wk{i}", name=f"wk_s{i}")
                    for i in range(c_t)]
            wv_s = [p_w.tile([128, f], BF16, tag=f"wv{i}", name=f"wv_s{i}")
                    for i in range(c_t)]
            bqs = p_sm.tile([128, f_t], F32, tag="bqs", name="bqs")
            bks = p_sm.tile([128, f_t], F32, tag="bks", name="bks")
            bvs = p_sm.tile([1, f], F32, tag="bvs", name="bvs")
            nc.sync.dma_start(bqs[:, :], bq.rearrange("(a p) o -> p (a o)", p=128))
            nc.sync.dma_start(bks[:, :], bk.rearrange("(a p) o -> p (a o)", p=128))
            nc.sync.dma_start(bvs[:, :], bv[:, :])
            for i in range(c_t):
                nc.sync.dma_start(xts[i][:, :], xT[i * 128:(i + 1) * 128, :])
                nc.sync.dma_start(wq_s[i][:, :], wq[i * 128:(i + 1) * 128, :])
                nc.sync.dma_start(wk_s[i][:, :], wk[i * 128:(i + 1) * 128, :])
            for i in range(c_t):
                nc.sync.dma_start(wv_s[i][:, :], wv[i * 128:(i + 1) * 128, :])
            wo_s = [p_wo.tile([128, cin], BF16, tag=f"wo{i}", name=f"wo_s{i}")
                    for i in range(f_t)]
            for i in range(f_t):
                nc.sync.dma_start(wo_s[i][:, :], wo[i * 128:(i + 1) * 128, :])

            onesf = p_sm.tile([128, nh], BF16, tag="onesf", name="onesf")
            nc.vector.memset(onesf[:, :], 1.0)
            # bv broadcast over all token partitions (used in V65 cast)
            bvs_bc = p_sm.tile([128, f], F32, tag="bvsbc", name="bvs_bc")
            nc.gpsimd.partition_broadcast(bvs_bc[:, :], bvs[:, :])

            QT = [p_qk.tile([128, tok], BF16, tag=f"qt{i}", name=f"QT{i}")
                  for i in range(f_t)]
            KT = [p_qk.tile([128, tok], BF16, tag=f"kt{i}", name=f"KT{i}")
                  for i in range(f_t)]
            V65 = [p_v.tile([128, nh * 65], BF16, tag=f"v{i}", name=f"V65_{i}")
                   for i in range(t_t)]
            attnT = [p_at.tile([128, tok], BF16, tag=f"at{i}", name=f"attnT{i}")
                     for i in range(f_t)]

            def emit_qk(ft, pool=None, ptag="pj"):
                """QT[ft], KT[ft] (feature-major) over all token chunks."""
                pool = pool or ps_pj
                for tch in range(tok // 512):
                    ts = slice(tch * 512, (tch + 1) * 512)
                    for (w_s, dst, bias) in ((wq_s, QT, bqs), (wk_s, KT, bks)):
                        ps = pool.tile([128, 512], F32, tag=ptag,
                                       name=f"psqk{ft}_{tch}_{dst[ft].name}")
                        for i in range(c_t):
                            nc.tensor.matmul(
                                ps[:, :],
                                w_s[i][:, ft * 128:(ft + 1) * 128],
                                xts[i][:, ts],
                                start=(i == 0), stop=(i == c_t - 1))
                        nc.vector.tensor_scalar_add(
                            dst[ft][:, ts], ps[:, :], bias[:, ft:ft + 1])

            def emit_v(gt, pool=None, ptag="pv"):
                """V65[gt]: token-major V + bias + ones column, bf16."""
                pool = pool or ps_pv
                tsl = slice(gt * 128, (gt + 1) * 128)
                psv = pool.tile([128, f], F32, tag=ptag, name=f"psv{gt}")
                for i in range(c_t):
                    nc.tensor.matmul(
                        psv[:, :], xts[i][:, tsl], wv_s[i][:, :],
                        start=(i == 0), stop=(i == c_t - 1))
                v_dst = V65[gt].rearrange("p (h e) -> p h e", e=65)
                nc.vector.tensor_copy(v_dst[:, :, 64:65], onesf[:, 0:nh])
                nc.vector.tensor_add(
                    v_dst[:, :, 0:64],
                    psv.rearrange("p (h e) -> p h e", e=64)[:, :, :],
                    bvs_bc.rearrange("p (h e) -> p h e", e=64)[:, :, :])

            def emit_out_proj(tt, pool=None, ptag="pj"):
                """Output projection for token tile tt (128 tokens)."""
                pool = pool or ps_pj
                tsl = slice(tt * 128, (tt + 1) * 128)
                for oc in range(n_oc):
                    osl = slice(oc * ocw, (oc + 1) * ocw)
                    po = pool.tile([128, ocw], F32, tag=ptag,
                                   name=f"po{tt}_{oc}")
                    for i in range(f_t):
                        nc.tensor.matmul(po[:, :], attnT[i][:, tsl],
                                         wo_s[i][:, osl],
                                         start=(i == 0), stop=(i == f_t - 1))
                    ob = p_os.tile([128, ocw], F32, tag="os",
                                   name=f"ob{tt}_{oc}")
                    nc.vector.tensor_copy(ob[:, :], po[:, :])
                    nc.sync.dma_start(o_part[tsl, osl], ob[:, :])

            def emit_attn(ft):
                """Attention for head pair ft (heads 2ft, 2ft+1)."""
                hA, hB = 2 * ft, 2 * ft + 1
                for qc in range(n_qc):
                    qs = slice(qc * 512, (qc + 1) * 512)
                    pvA = ps_pv.tile([65, 512], F32, tag="pv",
                                     name=f"pvA{ft}_{qc}")
                    pvB = ps_pv.tile([65, 512], F32, tag="pv",
                                     name=f"pvB{ft}_{qc}")
                    for jt in range(t_t):
                        if ft == 0 and qc == 0:
                            # race V65 production ahead of its consumption
                            if jt % 2 == 0:
                                emit_v(jt, ps_pj, "pj")
                            else:
                                emit_v(jt, ps_pv, "pv")
                        js = slice(jt * 128, (jt + 1) * 128)
                        sc = ps_sc.tile([128, 1024], F32, tag="sc",
                                        name=f"sc{ft}_{qc}_{jt}")
                        # paired score matmuls in disjoint PE row groups
                        nc.tensor.matmul(sc[:, 0:512],
                                         KT[ft][0:64, js], QT[ft][0:64, qs],
                                         start=True, stop=True)
                        nc.tensor.matmul(sc[:, 512:1024],
                                         KT[ft][64:128, js], QT[ft][64:128, qs],
                                         start=True, stop=True)
                        ex = p_ex.tile([128, 1024], BF16, tag="ex",
                                       name=f"ex{ft}_{qc}_{jt}")
                        nc.scalar.activation(ex[:, :], sc[:, :], AF.Exp,
                                             scale=SCALE)
                        nc.tensor.matmul(pvA[:, :],
                                         V65[jt][:, hA * 65:(hA + 1) * 65],
                                         ex[:, 0:512],
                                         start=(jt == 0), stop=(jt == t_t - 1))
                        nc.tensor.matmul(pvB[:, :],
                                         V65[jt][:, hB * 65:(hB + 1) * 65],
                                         ex[:, 512:1024],
                                         start=(jt == 0), stop=(jt == t_t - 1))
                    for (pv, r0) in ((pvA, 0), (pvB, 64)):
                        dn = p_dn.tile([1, 512], F32, tag="dn",
                                       name=f"dn{ft}_{qc}_{r0}")
                        nc.vector.tensor_copy(dn[:, :], pv[64:65, :])
                        dninv = p_dn.tile([1, 512], F32, tag="dninv",
                                          name=f"dninv{ft}_{qc}_{r0}")
                        nc.vector.reciprocal_approx_fast(
                            out=dninv[:, :], in_=dn[:, :])
                        rpb = p_rb.tile([64, 512], F32, tag="rpb",
                                        name=f"rpb{ft}_{qc}_{r0}")
                        nc.gpsimd.partition_broadcast(rpb[:, :], dninv[:, :])
                        nc.vector.tensor_mul(attnT[ft][r0:r0 + 64, qs],
                                             pv[0:64, :], rpb[:, :])
                    if ft == f_t - 1:
                        # all heads done for these 512 tokens: project out
                        for tt in range(qc * 4, (qc + 1) * 4):
                            if qc == n_qc - 1:
                                emit_out_proj(tt, ps_sc, "sc")
                            else:
                                emit_out_proj(tt)

            # ---- emission order: QK0 first (on the idle sc slots) so
            # attention starts early; V65 raced inside attn f0/qc0; QK[f+1]
            # threads into attention-f PE gaps on the pj slot.
            emit_qk(0, ps_sc, "sc")
            for ft in range(f_t):
                emit_attn(ft)
                if ft + 1 < f_t:
                    emit_qk(ft + 1)

    nc.finalize()
    return nc


_NC_CACHE = {}


def _get_nc(key=(N, C, NH)):
    if key not in _NC_CACHE:
        _NC_CACHE[key] = build_attention_kernel(*key)
    return _NC_CACHE[key]


def make_in_maps(x, Wq, bq, Wk, bk, Wv, bv, Wo):
    """Shard full inputs into 8 per-core input maps (bf16 operands)."""
    bf = ml_dtypes.bfloat16
    in_maps = []
    for c in range(NCORES):
        b, hg = divmod(c, 2)
        fs = slice(hg * F, (hg + 1) * F)
        in_maps.append({
            "xT": np.ascontiguousarray(x[b].T.astype(bf)),
            "wq": np.ascontiguousarray(Wq[:, fs].astype(bf)),
            "wk": np.ascontiguousarray(Wk[:, fs].astype(bf)),
            "wv": np.ascontiguousarray(Wv[:, fs].astype(bf)),
            "bq": np.ascontiguousarray(bq[fs].reshape(F, 1)),
            "bk": np.ascontiguousarray(bk[fs].reshape(F, 1)),
            "bv": np.ascontiguousarray(bv[fs].reshape(1, F)),
            "wo": np.ascontiguousarray(Wo[fs, :].astype(bf)),
        })
    return in_maps


def kernel(x, Wq, bq, Wk, bk, Wv, bv, Wo, bo, **_unused):
    from concourse.bass_utils import run_bass_kernel_spmd

    arrs = [np.asarray(a, dtype=np.float32)
            for a in (x, Wq, bq, Wk, bk, Wv, bv, Wo, bo)]
    x, Wq, bq, Wk, bk, Wv, bv, Wo, bo = arrs

    nc = _get_nc()
    in_maps = make_in_maps(x, Wq, bq, Wk, bk, Wv, bv, Wo)
    res = run_bass_kernel_spmd(nc, in_maps, core_ids=list(range(NCORES)))

    out = np.empty((B, N, C), dtype=np.float32)
    for b in range(B):
        out[b] = res.results[2 * b]["o_part"] + res.results[2 * b + 1]["o_part"] + bo
    return out
